# revision 1
# baseline (speedup 1.0000x reference)
"""Trainium2 Bass kernel for nn_BakaMega (EMA / damped cumulative conv).

Math: the reference's FFT causal cross-correlation with kernel
K[s,h] = alpha_h * q_h^(S-1-s), q_h = (1-alpha_h)*sigmoid(d1_h) is exactly
the first-order linear recurrence

    z[t] = q * z[t-1] + x[t];   y[t] = alpha * z[t]

per (batch, channel). On-device per core (H sharded 8 ways):
  - DMA x[b] natural layout -> SBUF tiles [128 seq x 128 ch]
  - TensorE transpose 128x128 blocks -> PSUM  (seq onto the free dim)
  - VectorE tensor_tensor_scan (state = q*state + x) straight from PSUM
  - TensorE matmul with diag(alpha) stationary-side: transposes back to
    natural layout AND applies alpha in the same pass
  - ScalarE copies PSUM->SBUF, DMA out.
"""

import numpy as np

from concourse import bacc, bass, mybir
from concourse.tile import TileContext
from concourse.masks import make_identity
from concourse.bass_utils import run_bass_kernel_spmd

B, S, H = 4, 4096, 2048
NCORES = 8
HC = H // NCORES        # 256 channels per core
P = 128                 # partitions
JBLK = S // P           # 32 seq blocks
F32 = mybir.dt.float32

_CACHE = {}


def _build_bass(reps=1, gblk=8, out_mode="amatmul", io_bufs=2, dma_halves=2,
                io_layout="per_b", mode="full", work_bufs=2):
    """gblk: transposes per PSUM group (psum tile = gblk*128 fp32 wide).
    out_mode: 'amatmul' (alpha-diag matmul) or 'transpose' (is_transpose +
    DVE alpha pre-scale on Y).
    dma_halves: split each per-batch 4MB DMA into this many seq-chunks."""
    nc = bacc.Bacc("TRN2", target_bir_lowering=False)
    x_d = nc.dram_tensor("x", [B, S, HC], F32, kind="ExternalInput")
    aux_d = nc.dram_tensor("aux", [HC, 2], F32, kind="ExternalInput")
    y_d = nc.dram_tensor("y", [B, S, HC], F32, kind="ExternalOutput")

    with TileContext(nc) as tc:
        n_groups = JBLK // gblk
        psum_bufs = max(1, 4 // max(1, gblk // 4))  # half of PSUM per path
        with (
            tc.tile_pool(name="consts", bufs=1) as consts,
            tc.tile_pool(name="io", bufs=io_bufs) as io_pool,
            tc.tile_pool(name="work", bufs=work_bufs) as work,
            tc.tile_pool(name="psum", bufs=psum_bufs, space="PSUM") as psum,
        ):
            ident_g = consts.tile([P, P], F32)
            make_identity(nc, ident_g)

            # aux[c, 0] = q_c, aux[c, 1] = alpha_c; load channel-major so the
            # per-channel scalars land one-per-partition.
            auxt = consts.tile([P, 2, 2], F32)
            nc.sync.dma_start(auxt[:], aux_d.rearrange("(cb p) k -> p cb k", p=P))

            # Funnel cross-engine deps through single DVE copies so derived
            # constants only depend on DVE program order (walrus limits the
            # sync-wait slots per instruction).
            ident = consts.tile([P, P], F32)
            nc.vector.tensor_copy(ident[:], ident_g[:])
            auxv = consts.tile([P, 2, 2], F32)
            nc.vector.tensor_copy(auxv[:], auxt[:])

            # qb[cb]: q broadcast along the free dim for the scan's data0.
            qb = []
            adiag = []
            qbw = gblk * P  # scan's data0 only needs one psum-group width
            for cb in range(2):
                t = consts.tile([P, qbw], F32, tag=f"qb{cb}")
                nc.vector.memset(t[:], 1.0)
                nc.vector.tensor_scalar_mul(t[:], t[:], auxv[:, cb, 0:1])
                qb.append(t)
                d = consts.tile([P, P], F32, tag=f"adiag{cb}")
                nc.vector.tensor_scalar_mul(d[:], ident[:], auxv[:, cb, 1:2])
                adiag.append(d)

            for rep in range(reps):
                for b in range(B):
                    src_b = x_d[b].rearrange("(j p) c -> p j c", p=P)
                    dst_b = y_d[b].rearrange("(j p) c -> p j c", p=P)
                    jh = JBLK // dma_halves
                    if io_layout == "per_b":
                        # full 1KB channel rows, one L2/O2 pair per batch
                        L2 = io_pool.tile([P, JBLK, HC], F32, tag="L2")
                        if mode != "compute_only":
                            for h in range(dma_halves):
                                nc.sync.dma_start(
                                    L2[:, h * jh : (h + 1) * jh, :],
                                    src_b[:, h * jh : (h + 1) * jh, :],
                                )
                        O2 = io_pool.tile([P, JBLK, HC], F32, tag="O2")
                    if mode == "dma_only":
                        for h in range(dma_halves):
                            nc.sync.dma_start(
                                dst_b[:, h * jh : (h + 1) * jh, :],
                                L2[:, h * jh : (h + 1) * jh, :],
                            )
                        continue
                    for cb in range(2):
                        if io_layout == "per_b":
                            L = L2[:, :, cb * P : (cb + 1) * P]
                        else:
                            Lt = io_pool.tile([P, JBLK, P], F32, tag="L")
                            for h in range(dma_halves):
                                nc.sync.dma_start(
                                    Lt[:, h * jh : (h + 1) * jh, :],
                                    src_b[:, h * jh : (h + 1) * jh,
                                          cb * P : (cb + 1) * P],
                                )
                            L = Lt[:]

                        if io_layout != "per_b":
                            O = io_pool.tile([P, JBLK, P], F32, tag="O")
                        Y = work.tile([P, S], F32, tag="Y")
                        GW = gblk * P  # free elems per psum group
                        for g in range(n_groups):
                            pin = psum.tile([P, GW], F32, tag="pin")
                            for jj in range(gblk):
                                j = g * gblk + jj
                                nc.tensor.transpose(
                                    pin[:, jj * P : (jj + 1) * P],
                                    L[:, j, :],
                                    ident[:],
                                )
                            init = 0.0 if g == 0 else Y[:, g * GW - 1 : g * GW]
                            nc.vector.tensor_tensor_scan(
                                Y[:, g * GW : (g + 1) * GW],
                                qb[cb][:, 0:GW],
                                pin[:],
                                init,
                                mybir.AluOpType.mult,
                                mybir.AluOpType.add,
                            )

                        if out_mode == "transpose":
                            # fold alpha into Y, then plain transposes back
                            nc.vector.tensor_scalar_mul(
                                Y[:], Y[:], auxv[:, cb, 1:2]
                            )

                        for g in range(n_groups):
                            pout = psum.tile([P, GW], F32, tag="pout")
                            for jj in range(gblk):
                                j = g * gblk + jj
                                if out_mode.startswith("transpose"):
                                    nc.tensor.transpose(
                                        pout[:, jj * P : (jj + 1) * P],
                                        Y[:, j * P : (j + 1) * P],
                                        ident[:],
                                    )
                                else:
                                    # out[s, c] = sum_k Y[k, 128j+s]*adiag[k, c]
                                    #           = alpha_c * Y[c, 128j+s]
                                    nc.tensor.matmul(
                                        pout[:, jj * P : (jj + 1) * P],
                                        Y[:, j * P : (j + 1) * P],
                                        adiag[cb][:],
                                    )
                            if io_layout == "per_b":
                                o_dst = O2[:, g * gblk : (g + 1) * gblk,
                                           cb * P : (cb + 1) * P]
                            else:
                                o_dst = O[:, g * gblk : (g + 1) * gblk, :]
                            nc.scalar.activation(
                                o_dst,
                                pout[:].rearrange("p (j c) -> p j c", c=P),
                                mybir.ActivationFunctionType.Copy,
                            )

                        if io_layout != "per_b":
                            for h in range(dma_halves):
                                nc.sync.dma_start(
                                    dst_b[:, h * jh : (h + 1) * jh,
                                          cb * P : (cb + 1) * P],
                                    O[:, h * jh : (h + 1) * jh, :],
                                )

                    if io_layout == "per_b" and mode != "compute_only":
                        for h in range(dma_halves):
                            nc.sync.dma_start(
                                dst_b[:, h * jh : (h + 1) * jh, :],
                                O2[:, h * jh : (h + 1) * jh, :],
                            )
    nc.finalize()
    return nc


def get_nc(reps=1, **kw):
    key = ("nc", reps, tuple(sorted(kw.items())))
    if key not in _CACHE:
        _CACHE[key] = _build_bass(reps, **kw)
    return _CACHE[key]


def _in_maps(x, dampeners):
    d = dampeners.astype(np.float64)
    alpha = 1.0 / (1.0 + np.exp(-d[0]))
    q = (1.0 - alpha) / (1.0 + np.exp(-d[1]))
    maps = []
    for c in range(NCORES):
        sl = slice(c * HC, (c + 1) * HC)
        aux = np.stack(
            [q[sl].astype(np.float32), alpha[sl].astype(np.float32)], axis=1
        )  # [HC, 2]
        maps.append(
            {
                "x": np.ascontiguousarray(x[:, :, sl]),
                "aux": np.ascontiguousarray(aux),
            }
        )
    return maps


def run(x, dampeners, reps=1, build_kw=None, **spmd_kwargs):
    nc = get_nc(reps, **(build_kw or {}))
    res = run_bass_kernel_spmd(
        nc, _in_maps(x, dampeners), list(range(NCORES)), **spmd_kwargs
    )
    y = np.concatenate([r["y"] for r in res.results], axis=2)
    return y.astype(np.float32), res


def kernel(x, dampeners):
    y, _ = run(x, dampeners)
    return y



# revision 5
# speedup vs baseline: 2.5156x; 2.5156x over previous
"""Trainium2 Bass kernel for nn_BakaMega (EMA / damped cumulative conv).

Math: the reference's FFT causal cross-correlation with kernel
K[s,h] = alpha_h * q_h^(S-1-s), q_h = (1-alpha_h)*sigmoid(d1_h) is exactly
the first-order linear recurrence

    y[t] = q * y[t-1] + alpha * x[t]

per (batch, channel), i.e. a causal exponential FIR y[t] = sum_d k[d] x[t-d]
with k[d] = alpha * q^d.

Fast path (dampeners channel-uniform, which holds for the nn.Parameter init
[[0.9999],[0.9899]].repeat_interleave): q ~ 0.196, so k decays below fp
noise within ~32 taps and the conv maps onto TensorE matmuls in the
NATURAL data layout (seq-within-block on partitions = contraction dim):

    y_block[j] = T1.T @ x_block[j] + T2.T @ x_block[j-1]
    T1[s,t] = k[t-s] (t>=s), T2[s,t] = k[128+t-s]

No transposes, no scan. Everything is fp16 I/O (tolerance 2e-2; measured
rel err ~3e-4): halves DMA and quadruples PE throughput vs fp32.
Per core (H sharded 8 ways): DMA x[b] natural -> [128 seq x (j,c)] tiles,
2 matmuls per 2-block pair into one PSUM bank, ScalarE/VectorE alternate
PSUM->SBUF eviction with fp32->fp16 cast, DMA out.

Fallback path (general per-channel dampeners or q -> 1): the original
exact tensor_tensor_scan kernel.
"""

import numpy as np

from concourse import bacc, bass, mybir
from concourse.tile import TileContext
from concourse.masks import make_identity
from concourse.bass_utils import run_bass_kernel_spmd

B, S, H = 4, 4096, 2048
NCORES = 8
HC = H // NCORES        # 256 channels per core
P = 128                 # partitions
JBLK = S // P           # 32 seq blocks
NPAIR = JBLK // 2       # 16 block pairs
F32 = mybir.dt.float32
F16 = mybir.dt.float16

_CACHE = {}


def _build_fir(reps=1, io_bufs=2, dma_halves=2, psum_bufs=8, evac="alt",
               mode="full"):
    """FIR fast path: block-banded matmuls in natural layout, fp16 I/O."""
    nc = bacc.Bacc("TRN2", target_bir_lowering=False)
    x_d = nc.dram_tensor("x", [B, S, HC], F16, kind="ExternalInput")
    w_d = nc.dram_tensor("w", [2, P, P], F16, kind="ExternalInput")
    y_d = nc.dram_tensor("y", [B, S, HC], F16, kind="ExternalOutput")

    with TileContext(nc) as tc:
        with (
            tc.tile_pool(name="consts", bufs=1) as consts,
            tc.tile_pool(name="xin", bufs=io_bufs) as xin,
            tc.tile_pool(name="yout", bufs=io_bufs) as yout,
            tc.tile_pool(name="psum", bufs=psum_bufs, space="PSUM") as psum,
        ):
            wt = consts.tile([P, 2, P], F16)
            nc.sync.dma_start(wt[:], w_d.rearrange("k p t -> p k t"))
            w1 = wt[:, 0, :]
            w2 = wt[:, 1, :]

            jh = JBLK // dma_halves
            for rep in range(reps):
                for b in range(B):
                    src_b = x_d[b].rearrange("(j p) c -> p j c", p=P)
                    dst_b = y_d[b].rearrange("(j p) c -> p j c", p=P)
                    L = xin.tile([P, JBLK, HC], F16, tag="L")
                    if mode != "compute_only":
                        for h in range(dma_halves):
                            nc.sync.dma_start(
                                L[:, h * jh : (h + 1) * jh, :],
                                src_b[:, h * jh : (h + 1) * jh, :],
                            )
                    O = yout.tile([P, JBLK, HC], F16, tag="O")
                    if mode == "dma_only":
                        for h in range(dma_halves):
                            nc.sync.dma_start(
                                dst_b[:, h * jh : (h + 1) * jh, :],
                                L[:, h * jh : (h + 1) * jh, :],
                            )
                        continue
                    for pi in range(NPAIR):
                        PT = psum.tile([P, 2 * HC], F32, tag="pt")
                        if pi == 0:
                            # block 0 has no predecessor: T1 only
                            nc.tensor.matmul(
                                PT[:, 0:HC], w1, L[:, 0, :], start=True, stop=True
                            )
                            nc.tensor.matmul(
                                PT[:, HC:], w1, L[:, 1, :], start=True, stop=False
                            )
                            nc.tensor.matmul(
                                PT[:, HC:], w2, L[:, 0, :], start=False, stop=True
                            )
                        else:
                            nc.tensor.matmul(
                                PT[:],
                                w1,
                                L[:, 2 * pi : 2 * pi + 2, :],
                                start=True,
                                stop=False,
                            )
                            nc.tensor.matmul(
                                PT[:],
                                w2,
                                L[:, 2 * pi - 1 : 2 * pi + 1, :],
                                start=False,
                                stop=True,
                            )
                        o_dst = O[:, 2 * pi : 2 * pi + 2, :]
                        src = PT[:].rearrange("p (j c) -> p j c", c=HC)
                        if evac == "alt" and pi % 2 == 0:
                            nc.scalar.activation(
                                o_dst, src, mybir.ActivationFunctionType.Copy
                            )
                        else:
                            nc.vector.tensor_copy(o_dst, src)
                    if mode != "compute_only":
                        for h in range(dma_halves):
                            nc.sync.dma_start(
                                dst_b[:, h * jh : (h + 1) * jh, :],
                                O[:, h * jh : (h + 1) * jh, :],
                            )
    nc.finalize()
    return nc


def _build_bass(reps=1, gblk=8, out_mode="amatmul", io_bufs=2, dma_halves=2,
                io_layout="per_b", mode="full", work_bufs=2):
    """Exact per-channel scan path (fallback). gblk: transposes per PSUM
    group. out_mode: 'amatmul' (alpha-diag matmul) or 'transpose'."""
    nc = bacc.Bacc("TRN2", target_bir_lowering=False)
    x_d = nc.dram_tensor("x", [B, S, HC], F32, kind="ExternalInput")
    aux_d = nc.dram_tensor("aux", [HC, 2], F32, kind="ExternalInput")
    y_d = nc.dram_tensor("y", [B, S, HC], F32, kind="ExternalOutput")

    with TileContext(nc) as tc:
        n_groups = JBLK // gblk
        psum_bufs = max(1, 4 // max(1, gblk // 4))  # half of PSUM per path
        with (
            tc.tile_pool(name="consts", bufs=1) as consts,
            tc.tile_pool(name="io", bufs=io_bufs) as io_pool,
            tc.tile_pool(name="work", bufs=work_bufs) as work,
            tc.tile_pool(name="psum", bufs=psum_bufs, space="PSUM") as psum,
        ):
            ident_g = consts.tile([P, P], F32)
            make_identity(nc, ident_g)

            # aux[c, 0] = q_c, aux[c, 1] = alpha_c; load channel-major so the
            # per-channel scalars land one-per-partition.
            auxt = consts.tile([P, 2, 2], F32)
            nc.sync.dma_start(auxt[:], aux_d.rearrange("(cb p) k -> p cb k", p=P))

            # Funnel cross-engine deps through single DVE copies so derived
            # constants only depend on DVE program order (walrus limits the
            # sync-wait slots per instruction).
            ident = consts.tile([P, P], F32)
            nc.vector.tensor_copy(ident[:], ident_g[:])
            auxv = consts.tile([P, 2, 2], F32)
            nc.vector.tensor_copy(auxv[:], auxt[:])

            # qb[cb]: q broadcast along the free dim for the scan's data0.
            qb = []
            adiag = []
            qbw = gblk * P  # scan's data0 only needs one psum-group width
            for cb in range(2):
                t = consts.tile([P, qbw], F32, tag=f"qb{cb}")
                nc.vector.memset(t[:], 1.0)
                nc.vector.tensor_scalar_mul(t[:], t[:], auxv[:, cb, 0:1])
                qb.append(t)
                d = consts.tile([P, P], F32, tag=f"adiag{cb}")
                nc.vector.tensor_scalar_mul(d[:], ident[:], auxv[:, cb, 1:2])
                adiag.append(d)

            for rep in range(reps):
                for b in range(B):
                    src_b = x_d[b].rearrange("(j p) c -> p j c", p=P)
                    dst_b = y_d[b].rearrange("(j p) c -> p j c", p=P)
                    jh = JBLK // dma_halves
                    if io_layout == "per_b":
                        # full 1KB channel rows, one L2/O2 pair per batch
                        L2 = io_pool.tile([P, JBLK, HC], F32, tag="L2")
                        if mode != "compute_only":
                            for h in range(dma_halves):
                                nc.sync.dma_start(
                                    L2[:, h * jh : (h + 1) * jh, :],
                                    src_b[:, h * jh : (h + 1) * jh, :],
                                )
                        O2 = io_pool.tile([P, JBLK, HC], F32, tag="O2")
                    if mode == "dma_only":
                        for h in range(dma_halves):
                            nc.sync.dma_start(
                                dst_b[:, h * jh : (h + 1) * jh, :],
                                L2[:, h * jh : (h + 1) * jh, :],
                            )
                        continue
                    for cb in range(2):
                        if io_layout == "per_b":
                            L = L2[:, :, cb * P : (cb + 1) * P]
                        else:
                            Lt = io_pool.tile([P, JBLK, P], F32, tag="L")
                            for h in range(dma_halves):
                                nc.sync.dma_start(
                                    Lt[:, h * jh : (h + 1) * jh, :],
                                    src_b[:, h * jh : (h + 1) * jh,
                                          cb * P : (cb + 1) * P],
                                )
                            L = Lt[:]

                        if io_layout != "per_b":
                            O = io_pool.tile([P, JBLK, P], F32, tag="O")
                        Y = work.tile([P, S], F32, tag="Y")
                        GW = gblk * P  # free elems per psum group
                        for g in range(n_groups):
                            pin = psum.tile([P, GW], F32, tag="pin")
                            for jj in range(gblk):
                                j = g * gblk + jj
                                nc.tensor.transpose(
                                    pin[:, jj * P : (jj + 1) * P],
                                    L[:, j, :],
                                    ident[:],
                                )
                            init = 0.0 if g == 0 else Y[:, g * GW - 1 : g * GW]
                            nc.vector.tensor_tensor_scan(
                                Y[:, g * GW : (g + 1) * GW],
                                qb[cb][:, 0:GW],
                                pin[:],
                                init,
                                mybir.AluOpType.mult,
                                mybir.AluOpType.add,
                            )

                        if out_mode == "transpose":
                            # fold alpha into Y, then plain transposes back
                            nc.vector.tensor_scalar_mul(
                                Y[:], Y[:], auxv[:, cb, 1:2]
                            )

                        for g in range(n_groups):
                            pout = psum.tile([P, GW], F32, tag="pout")
                            for jj in range(gblk):
                                j = g * gblk + jj
                                if out_mode.startswith("transpose"):
                                    nc.tensor.transpose(
                                        pout[:, jj * P : (jj + 1) * P],
                                        Y[:, j * P : (j + 1) * P],
                                        ident[:],
                                    )
                                else:
                                    # out[s, c] = sum_k Y[k, 128j+s]*adiag[k, c]
                                    #           = alpha_c * Y[c, 128j+s]
                                    nc.tensor.matmul(
                                        pout[:, jj * P : (jj + 1) * P],
                                        Y[:, j * P : (j + 1) * P],
                                        adiag[cb][:],
                                    )
                            if io_layout == "per_b":
                                o_dst = O2[:, g * gblk : (g + 1) * gblk,
                                           cb * P : (cb + 1) * P]
                            else:
                                o_dst = O[:, g * gblk : (g + 1) * gblk, :]
                            nc.scalar.activation(
                                o_dst,
                                pout[:].rearrange("p (j c) -> p j c", c=P),
                                mybir.ActivationFunctionType.Copy,
                            )

                        if io_layout != "per_b":
                            for h in range(dma_halves):
                                nc.sync.dma_start(
                                    dst_b[:, h * jh : (h + 1) * jh,
                                          cb * P : (cb + 1) * P],
                                    O[:, h * jh : (h + 1) * jh, :],
                                )

                    if io_layout == "per_b" and mode != "compute_only":
                        for h in range(dma_halves):
                            nc.sync.dma_start(
                                dst_b[:, h * jh : (h + 1) * jh, :],
                                O2[:, h * jh : (h + 1) * jh, :],
                            )
    nc.finalize()
    return nc


def get_nc(reps=1, path="fir", **kw):
    key = ("nc", path, reps, tuple(sorted(kw.items())))
    if key not in _CACHE:
        builder = _build_fir if path == "fir" else _build_bass
        _CACHE[key] = builder(reps, **kw)
    return _CACHE[key]


def _alpha_q(dampeners):
    d = dampeners.astype(np.float64)
    alpha = 1.0 / (1.0 + np.exp(-d[0]))
    q = (1.0 - alpha) / (1.0 + np.exp(-d[1]))
    return alpha, q


def _pick_path(dampeners):
    d = np.asarray(dampeners, np.float64)
    _, q = _alpha_q(d)
    uniform = np.allclose(d, d[:, :1], rtol=0, atol=0)
    # FIR truncates cross-block history at 256 taps: needs q^128 ~ 0.
    if uniform and float(q.max()) < 0.9:
        return "fir"
    return "scan"


def _in_maps(x, dampeners):
    if _pick_path(dampeners) == "fir":
        alpha, q = _alpha_q(dampeners)
        a0, q0 = float(alpha[0]), float(q[0])
        s_ = np.arange(P, dtype=np.float64)[:, None]
        t_ = np.arange(P, dtype=np.float64)[None, :]
        T1 = np.where(t_ >= s_, a0 * q0 ** np.maximum(t_ - s_, 0.0), 0.0)
        T2 = a0 * q0 ** (128.0 + t_ - s_)
        w = np.stack([T1, T2]).astype(np.float16)  # [2, s, t]
        x16 = x.astype(np.float16)
        return [
            {"x": np.ascontiguousarray(x16[:, :, c * HC : (c + 1) * HC]), "w": w}
            for c in range(NCORES)
        ]
    alpha, q = _alpha_q(dampeners)
    maps = []
    for c in range(NCORES):
        sl = slice(c * HC, (c + 1) * HC)
        aux = np.stack(
            [q[sl].astype(np.float32), alpha[sl].astype(np.float32)], axis=1
        )  # [HC, 2]
        maps.append(
            {
                "x": np.ascontiguousarray(x[:, :, sl]),
                "aux": np.ascontiguousarray(aux),
            }
        )
    return maps


def run(x, dampeners, reps=1, build_kw=None, **spmd_kwargs):
    path = _pick_path(dampeners)
    nc = get_nc(reps, path=path, **(build_kw or {}))
    res = run_bass_kernel_spmd(
        nc, _in_maps(x, dampeners), list(range(NCORES)), **spmd_kwargs
    )
    y = np.concatenate([r["y"] for r in res.results], axis=2)
    return y.astype(np.float32), res


def kernel(x, dampeners):
    y, _ = run(x, dampeners)
    return y
